# revision 50
# baseline (speedup 1.0000x reference)
"""EHR memory-network kernel for Trainium2 (8 NeuronCores, data-parallel over batch).

Reformulation of the reference scatter-scan:
  For patient b the scan applies, per event e (in time order), the affine update
      M[id_e] = M[id_e] * Af[e] + Bf[e]
  Slot 0 is never touched (ids >= 1) and every touched slot starts from the same
  init_mem vector, so the final row for node n is
      M[n] = init_mem * prod_{e: id_e=n} Af[e] + sum_{e: id_e=n} Bf[e] * SufA[e]
  with SufA[e] = prod_{j>e, id_j=id_e} Af[j].

Key structural facts exploited on device:
  * Most (patient, node) groups are singletons (ids are near-unique): for those
    the final row is simply init*Af + Bf -- no cross-event combination at all.
  * The few collision groups (~40 events/core) are gathered into one 128-slot
    buffer via 0/1 selector matmuls on the PE; suffix products within the
    (host-sorted, contiguous) groups are computed by log2-step masked-shift
    matmuls + elementwise multiplies; the group sums are one compare-matrix
    matmul.  No ln/exp anywhere -> a single activation table load.
  * The D-level erase/add chain composes through host-built scaled-shift
    matrices on the PE (Mk/Ms land in PSUM directly, bias via rank-1 matmul),
    leaving only elementwise products/adds on DVE/Pool.  (Partition-offset
    SBUF reads are illegal on TRN2, so shifts must ride the PE.)
  * The 16MB init table write is spread over the three DMA-capable queues
    (SP/Activation/Pool): SP is a pure DMA lane, small blocks ride Act/Pool
    pipeline gaps, stragglers fill at the end.  All value math is fp16
    (PSUM accumulation in f32); the rel-err budget is 2e-2.

Host prep/finish is index-only: validity compaction, patient balancing, quad
(component) packing, selector/mask/shift matrix construction, fp16 repacks,
and final row placement out[id_e] = row[e].
"""

import math
import numpy as np
from contextlib import ExitStack

import concourse.bass as bass
import concourse.tile as tile
from concourse import bacc, mybir
from concourse import bass_utils

F32 = mybir.dt.float32
F16 = mybir.dt.float16
AF = mybir.ActivationFunctionType
OP = mybir.AluOpType

# Problem shapes (hardcoded per contest contract).
B, T, MOD, D = 32, 64, 3, 4
WD, MEM, HID, DEMO = 256, 256, 512, 64
N_NODES = 4096
N_CORES = 8
BP = B // N_CORES              # patients per core = 4
P = 128
NCH = 7                        # event chunks of 128 per core
S_C = NCH * P                  # event slots per core = 896
QUADS = S_C // D               # (t,mod) quads per core = 224
OUT_ROWS = BP * N_NODES        # 16384
RREP = 8                       # rows per partition per out block (1MB blocks)
ROWS_N = (NCH + 1) * P + BP    # 1028: 7 chunk slabs + gather slab + demo rows
# DMA block schedule knobs (1MB early SP blocks; MIDR-row blocks per chunk on
# Act/Pool mid-pipeline; round-robin 1MB tail fill)
SP_EARLY = 6
MIDR = 3                       # 0.25MB mid-pipeline blocks
ACT_MID = 2
POOL_MID = 2

# ---- wewaA layout (fp16, gate weights + chain shift matrices) ----
WA_G = 0                # 1024: We/2 || Wa interleaved per wd-half
WA_STH = 1024           # 3 x 128: scaled shift matrices for Mk (th half)
WA_SA = WA_STH + 384    # 3 x 128: scaled shift matrices for Ms (A half)
WA_BTH = WA_SA + 384    # 3 x 128: rank-1 bias rows (1+ca_k) on partition 0
WA_ONES = WA_BTH + 384  # 256: ones row on partition 0
WA_COLS = WA_ONES + 256

# ---- wewaB layout (fp16, gather/suffix machinery + demo) ----
WB_PSEL = 0            # 7 x 128: gather selector matrices
WB_G0 = WB_PSEL + NCH * P   # 128: collision same-group compare matrix
WB_W1 = WB_G0 + P      # 512: W1 (on partitions 0..63)
WB_W2P = WB_W1 + 512   # 256: W2 repacked
WB_W3B = WB_W2P + 256  # 256: W3||b3 (on partitions 0..64)
WB_DT = WB_W3B + 256   # 4: demoT (partitions 0..63)
WB_F32 = WB_DT + 4     # 16 fp16 cols = 8 f32: [b1(4), b2(1), spare]
WB_SC = WB_F32 + 16    # steps x (128 Sc matrix + 128 scbias row)
# WB_COLS depends on steps -> computed in _build_nc

_NC_CACHE = {}


def _build_nc(steps, ncc):
    nc = bacc.Bacc("TRN2", target_bir_lowering=False, debug=False,
                   enable_asserts=False, num_devices=N_CORES)
    wb_cols = WB_SC + 2 * steps * P
    t = {}
    t["initd"] = nc.dram_tensor("initd", [1, MEM], F32, kind="ExternalInput").ap()
    t["xT0"] = nc.dram_tensor("xT0", [P, S_C], F16, kind="ExternalInput").ap()
    t["xT1"] = nc.dram_tensor("xT1", [P, S_C], F16, kind="ExternalInput").ap()
    t["wewaA"] = nc.dram_tensor("wewaA", [P, WA_COLS], F16, kind="ExternalInput").ap()
    t["wewaB"] = nc.dram_tensor("wewaB", [P, wb_cols], F16, kind="ExternalInput").ap()
    t["out"] = nc.dram_tensor("out", [OUT_ROWS, MEM], F32, kind="ExternalOutput").ap()
    t["rows"] = nc.dram_tensor("rows", [ROWS_N, MEM], F16, kind="ExternalOutput").ap()

    with tile.TileContext(nc) as tc:
        with ExitStack() as ctx:
            _emit(ctx, tc, steps=steps, ncc=ncc, **t)
    nc.compile()
    return nc


def _emit(ctx, tc, *, steps, ncc, initd, xT0, xT1, wewaA, wewaB, out, rows):
    nc = tc.nc

    const = ctx.enter_context(tc.tile_pool(name="const", bufs=1))
    big = ctx.enter_context(tc.tile_pool(name="big", bufs=1))
    work = ctx.enter_context(tc.tile_pool(name="work", bufs=2))
    psum = ctx.enter_context(tc.tile_pool(name="psum", bufs=1, space="PSUM"))

    # ---------------- loads ----------------
    # Act queue: gate weights first (feeds the whole pipeline; the gates half
    # of wewaA goes in its own DMA so the first matmul can start sooner).
    wA = const.tile([P, WA_COLS], F16, tag="wA", name="wA")
    nc.scalar.dma_start(wA[:, 0:1024], wewaA[:, 0:1024])
    nc.scalar.dma_start(wA[:, 1024:WA_COLS], wewaA[:, 1024:WA_COLS])
    xT = [const.tile([P, S_C], F16, tag=f"xT{i}", name=f"xT{i}") for i in range(2)]
    # SP queue: both x halves (SP is otherwise the pure-DMA block lane).
    nc.sync.dma_start(xT[0][:], xT0[:])
    nc.sync.dma_start(xT[1][:], xT1[:])
    wb_cols = WB_SC + 2 * steps * P
    wB = const.tile([P, wb_cols], F16, tag="wB", name="wB")

    def wa(c0, n):
        return wA[:, c0:c0 + n]

    def wb(c0, n):
        return wB[:, c0:c0 + n]

    # ---------------- derived constants ----------------
    ones = const.tile([1, MEM], F16, tag="ones", name="ones")
    nc.vector.memset(ones[:], 1.0)
    # init_mem row: tiny dedicated load on Pool so initrep is ready early
    initr = const.tile([1, MEM], F32, tag="initr", name="initr")
    nc.gpsimd.dma_start(initr[:], initd[:])
    initrep = big.tile([P, RREP * MEM], F32, tag="initrep", name="initrep")
    nc.gpsimd.partition_broadcast(initrep[:, 0:MEM], initr[:])
    init16 = const.tile([P, MEM], F16, tag="init16", name="init16")
    nc.vector.tensor_copy(init16[:], initrep[:, 0:MEM])
    nc.gpsimd.tensor_copy(initrep[:, MEM:2 * MEM], initrep[:, 0:MEM])
    nc.vector.tensor_copy(initrep[:, 2 * MEM:4 * MEM], initrep[:, 0:2 * MEM])
    nc.gpsimd.tensor_copy(initrep[:, 4 * MEM:6 * MEM], initrep[:, 0:2 * MEM])
    nc.vector.tensor_copy(initrep[:, 6 * MEM:8 * MEM], initrep[:, 0:2 * MEM])

    # ---------------- the init-table block writes ----------------
    # SP is a pure DMA lane (emitted early, gated only by initrep); small
    # blocks ride the Act/Pool idle gaps in the chunk pipeline; leftovers
    # fill in round-robin at the end.
    row_cur = [0]

    def emit_blocks(eng, rrep, n=1):
        for _ in range(n):
            if row_cur[0] >= OUT_ROWS:
                return
            rrep = min(rrep, (OUT_ROWS - row_cur[0]) // P)
            r0_, r1_ = row_cur[0], row_cur[0] + P * rrep
            row_cur[0] = r1_
            dst = out[r0_:r1_, :].rearrange("(p r) m -> p (r m)", r=rrep)
            eng.dma_start(dst, initrep[:, 0:rrep * MEM])

    emit_blocks(nc.sync, 1)
    emit_blocks(nc.sync, 1)
    emit_blocks(nc.sync, 2)
    emit_blocks(nc.sync, 4)
    nc.sync.dma_start(wB[:], wewaB[:])
    emit_blocks(nc.sync, RREP, SP_EARLY)

    # ---------------- phase A: per-chunk gate/chain pipeline ----------------
    AlS = big.tile([P, NCH * MEM], F16, tag="AlS", name="AlS")
    BcS = big.tile([P, NCH * MEM], F16, tag="BcS", name="BcS")
    rowsAll = big.tile([P, (NCH + 1) * MEM], F16, tag="rowsAll", name="rowsAll")

    def cc(c, w=MEM):
        return slice(c * w, (c + 1) * w)

    # shared psum bank for phase B and the (late-emitted) demo block
    psD = psum.tile([P, 2 * MEM], F32, tag="psD", bufs=1, name="psD")
    # phase-B gather psums (pending only across the collision chunks)
    psAf = psum.tile([P, MEM], F32, tag="psAf", bufs=1, name="psAf")
    psBf = psum.tile([P, MEM], F32, tag="psBf", bufs=1, name="psBf")

    def emit_phase_b():
        # Collision-group suffix products / sums; overlaps the remaining
        # singleton chunks.  All psums live in psD (demo is done by now).
        Afg = work.tile([P, MEM], F16, tag="Afg", name="Afg")
        nc.vector.tensor_copy(Afg[:], psAf[:])
        Bfg = work.tile([P, MEM], F16, tag="Bfg", name="Bfg")
        nc.vector.tensor_copy(Bfg[:], psBf[:])
        W = Afg
        for s in range(steps):
            psW = psD[:, (s % 2) * MEM:(s % 2 + 1) * MEM]
            nc.tensor.matmul(psW, lhsT=wb(WB_SC + 2 * s * P, P),
                             rhs=W[:], start=True, stop=False)
            nc.tensor.matmul(psW, lhsT=wB[0:1, WB_SC + (2 * s + 1) * P:
                                          WB_SC + (2 * s + 2) * P],
                             rhs=ones[:], start=False, stop=True)
            Wn = work.tile([P, MEM], F16, tag=f"W{s}", name=f"W{s}")
            nc.vector.tensor_tensor(Wn[:], W[:], psW, op=OP.mult)
            W = Wn
        # W = INCLUSIVE group product from each slot: Pi_{j>=p} Af[j].
        # Exclusive suffix for the B sum = shift-by-1 of W (s=0 mask).
        psC = psD[:, (steps % 2) * MEM:(steps % 2 + 1) * MEM]
        nc.tensor.matmul(psC, lhsT=wb(WB_SC, P), rhs=W[:],
                         start=True, stop=False)
        nc.tensor.matmul(psC, lhsT=wB[0:1, WB_SC + P:WB_SC + 2 * P],
                         rhs=ones[:], start=False, stop=True)
        contrib = work.tile([P, MEM], F16, tag="contrib", name="contrib")
        nc.vector.tensor_tensor(contrib[:], Bfg[:], psC, op=OP.mult)
        psB = psD[:, ((steps + 1) % 2) * MEM:((steps + 1) % 2 + 1) * MEM]
        nc.tensor.matmul(psB, lhsT=wb(WB_G0, P), rhs=contrib[:],
                         start=True, stop=True)
        r0 = work.tile([P, MEM], F16, tag="r0", name="r0")
        nc.gpsimd.tensor_tensor(r0[:], W[:], init16[:], op=OP.mult)
        nc.vector.tensor_tensor(rowsAll[:, cc(NCH)], r0[:], psB, op=OP.add)

    for c in range(NCH):
        # gates: psEA = [tanh-arg of E (z/2) || tanh-arg of A] for 128 events
        psEA = psum.tile([P, 2 * MEM], F32, tag="psEA", bufs=1, name="psEA")
        for i in range(2):
            nc.tensor.matmul(psEA[:], lhsT=xT[i][:, cc(c, P)],
                             rhs=wa(WA_G + i * 512, 512),
                             start=(i == 0), stop=(i == 1))
        thA = work.tile([P, 2 * MEM], F16, tag="thA", bufs=3, name="thA")
        nc.scalar.activation(thA[:], psEA[:], AF.Tanh)
        th = thA[:, 0:MEM]
        A_ = thA[:, MEM:2 * MEM]

        # shift banks: bank_k = [Mk || Ms_k] directly in PSUM.
        # Emitted 3,2,1 and consumed in that order so bank2/bank3 free early
        # (bufs=1) while bank1, consumed last, is double-buffered.
        bank = {}
        for k in (3, 2, 1):
            ps = psum.tile([P, 2 * MEM], F32, tag=f"bk{k}",
                           bufs=(2 if k == 1 else 1), name=f"bk{k}")
            nc.tensor.matmul(ps[:, 0:MEM], lhsT=wa(WA_STH + (k - 1) * P, P),
                             rhs=th, start=True, stop=False)
            nc.tensor.matmul(ps[:, 0:MEM],
                             lhsT=wA[0:1, WA_BTH + (k - 1) * P:WA_BTH + k * P],
                             rhs=ones[:], start=False, stop=True)
            nc.tensor.matmul(ps[:, MEM:2 * MEM], lhsT=wa(WA_SA + (k - 1) * P, P),
                             rhs=A_, start=True, stop=True)
            bank[k] = ps
        M1, Ms1 = bank[1][:, 0:MEM], bank[1][:, MEM:2 * MEM]
        M2, Ms2 = bank[2][:, 0:MEM], bank[2][:, MEM:2 * MEM]
        M3, Ms3 = bank[3][:, 0:MEM], bank[3][:, MEM:2 * MEM]

        # Al = M0*M1*M2*M3, Bc = A*T1 + Ms1*T2 + Ms2*T3 + Ms3
        # (DVE owns every PSUM-reading op -- GPSIMD cannot touch PSUM -- and
        #  Pool takes the SBUF-only fp16 tail.)
        M0 = work.tile([P, MEM], F16, tag="M0", bufs=3, name="M0")
        nc.vector.tensor_scalar(M0[:], th, -0.5, 0.5, op0=OP.mult, op1=OP.add)
        T3 = work.tile([P, MEM], F16, tag="T3", bufs=3, name="T3")
        nc.vector.tensor_copy(T3[:], M3)
        T2 = work.tile([P, MEM], F16, tag="T2", bufs=3, name="T2")
        nc.vector.tensor_tensor(T2[:], M2, T3[:], op=OP.mult)
        u3 = work.tile([P, MEM], F16, tag="u3", bufs=3, name="u3")
        nc.vector.tensor_tensor(u3[:], Ms2, T3[:], op=OP.mult)
        v2 = work.tile([P, MEM], F16, tag="v2", bufs=3, name="v2")
        nc.vector.tensor_tensor(v2[:], u3[:], Ms3, op=OP.add)
        T1 = work.tile([P, MEM], F16, tag="T1", bufs=3, name="T1")
        nc.vector.tensor_tensor(T1[:], M1, T2[:], op=OP.mult)
        u2 = work.tile([P, MEM], F16, tag="u2", bufs=3, name="u2")
        nc.vector.tensor_tensor(u2[:], Ms1, T2[:], op=OP.mult)
        Al = AlS[:, cc(c)]
        nc.gpsimd.tensor_tensor(Al, M0[:], T1[:], op=OP.mult)
        # rt only needs Al: hoist it so the chunk closes on Bc -> rows alone
        rt = work.tile([P, MEM], F16, tag="rt", bufs=3, name="rt")
        nc.gpsimd.tensor_tensor(rt[:], Al, init16[:], op=OP.mult)
        u1 = work.tile([P, MEM], F16, tag="u1", bufs=3, name="u1")
        nc.gpsimd.tensor_tensor(u1[:], A_, T1[:], op=OP.mult)
        nc.gpsimd.tensor_tensor(u1[:], u1[:], u2[:], op=OP.add)
        Bc = BcS[:, cc(c)]
        nc.gpsimd.tensor_tensor(Bc, u1[:], v2[:], op=OP.add)
        nc.gpsimd.tensor_tensor(rowsAll[:, cc(c)], rt[:], Bc, op=OP.add)

        if c == 5:
            nc.gpsimd.dma_start(
                rows[0:5 * P, :].rearrange("(c p) m -> p c m", p=P),
                rowsAll[:, 0:5 * MEM].rearrange("p (c m) -> p c m", c=5))
        # phase-B gathers accumulate while the collision chunks land
        if c < ncc:
            nc.tensor.matmul(psAf[:], lhsT=wb(WB_PSEL + c * P, P), rhs=Al,
                             start=(c == 0), stop=(c == ncc - 1))
            nc.tensor.matmul(psBf[:], lhsT=wb(WB_PSEL + c * P, P), rhs=Bc,
                             start=(c == 0), stop=(c == ncc - 1))
        if c == ncc - 1:
            emit_phase_b()

        # small blocks ride Act/Pool idle gaps under the chain
        emit_blocks(nc.scalar, MIDR, ACT_MID)
        emit_blocks(nc.gpsimd, MIDR, POOL_MID)

    # ---------------- demographics residual block ----------------
    b1c = wb(WB_F32, 16).bitcast(F32)      # [128, 8] f32: b1 cols 0..3, b2 col 4
    demoT = wb(WB_DT, 4)[0:DEMO, :]        # [64, 4]
    hT = [work.tile([P, BP], F16, tag=f"hT{i}", name=f"hT{i}") for i in range(4)]
    for i in range(4):
        ps = psD[:, i * BP:(i + 1) * BP]
        nc.tensor.matmul(ps, lhsT=wb(WB_W1 + i * P, P)[0:DEMO, :],
                         rhs=demoT, start=True, stop=True)
        nc.scalar.activation(hT[i][:], ps, AF.Relu,
                             bias=b1c[:, i:i + 1], scale=1.0)
    ps_y = psD[0:DEMO, 16:16 + BP]
    for i in range(4):
        nc.tensor.matmul(ps_y, lhsT=wb(WB_W2P + i * DEMO, DEMO),
                         rhs=hT[i][:], start=(i == 0), stop=(i == 3))
    yTe = work.tile([DEMO + 1, BP], F16, tag="yTe", name="yTe")
    nc.vector.tensor_copy(yTe[DEMO:DEMO + 1, :], ones[:, 0:BP])
    # y = psy + b2 + demo  (b2 per-partition bias, demo residual)
    nc.scalar.activation(yTe[0:DEMO, :], ps_y, AF.Identity,
                         bias=b1c[0:DEMO, 4:5], scale=1.0)
    nc.vector.tensor_tensor(yTe[0:DEMO, :], yTe[0:DEMO, :], demoT, op=OP.add)
    psde = psD[0:BP, MEM:2 * MEM]
    nc.tensor.matmul(psde, lhsT=yTe[:], rhs=wb(WB_W3B, MEM)[0:DEMO + 1, :],
                     start=True, stop=True)
    de16 = work.tile([BP, MEM], F16, tag="de16", name="de16")
    nc.vector.tensor_copy(de16[:], psde)

    # ---------------- remaining block writes + rows ----------------
    tail = [nc.gpsimd, nc.scalar, nc.sync]
    ti = 0
    while row_cur[0] < OUT_ROWS:
        emit_blocks(tail[ti % 3], RREP)
        ti += 1
    nc.gpsimd.dma_start(
        rows[5 * P:NCH * P, :].rearrange("(c p) m -> p c m", p=P),
        rowsAll[:, 5 * MEM:NCH * MEM].rearrange("p (c m) -> p c m", c=NCH - 5))
    nc.scalar.dma_start(rows[NCH * P:(NCH + 1) * P, :], rowsAll[:, cc(NCH)])
    nc.scalar.dma_start(rows[(NCH + 1) * P:(NCH + 1) * P + BP, :], de16[:])


# ======================= host side =======================

def _assign_patients(gvalid):
    """Balanced 4-patients-per-core assignment by valid-quad count (LPT)."""
    counts = gvalid.reshape(B, -1).sum(1)
    order = np.argsort(-counts, kind="stable")
    loads = [0] * N_CORES
    members = [[] for _ in range(N_CORES)]
    for p in order:
        c = min((c for c in range(N_CORES) if len(members[c]) < BP),
                key=lambda c: loads[c])
        members[c].append(int(p))
        loads[c] += int(counts[p])
    assert max(loads) <= QUADS, f"core load {max(loads)} quads > {QUADS}"
    return members


def _prep_core(x, node_ids, gvalid_core, pats):
    """Pack one core: quad components -> chunks, collision gather, matrices."""
    # Enumerate valid quads: (slot, tm) with 4 events (d-levels) each.
    quads = []       # (slot, tm, ids[4])
    for slot, b in enumerate(pats):
        for tm in np.nonzero(gvalid_core[slot].reshape(T * MOD))[0]:
            ids = node_ids[b, tm // MOD, tm % MOD]   # [4]
            quads.append((slot, int(tm), ids))

    # Union-find over quads via shared (slot, id).
    parent = list(range(len(quads)))

    def find(a):
        while parent[a] != a:
            parent[a] = parent[parent[a]]
            a = parent[a]
        return a

    id2q = {}
    groups = {}   # (slot, id) -> list of (quad_idx, d)
    for qi, (slot, tm, ids) in enumerate(quads):
        for d in range(D):
            key = (slot, int(ids[d]))
            groups.setdefault(key, []).append((qi, d))
            if key in id2q:
                ra, rb = find(id2q[key]), find(qi)
                if ra != rb:
                    parent[rb] = ra
            else:
                id2q[key] = qi
    comps = {}
    for qi in range(len(quads)):
        comps.setdefault(find(qi), []).append(qi)
    has_coll = {r: False for r in comps}
    for key, members_ in groups.items():
        if len(members_) >= 2:
            has_coll[find(members_[0][0])] = True

    # First-fit pack of components into NCH bins of 32 quads; collision
    # components first so phase B only waits on the earliest chunk(s).
    bins = [[] for _ in range(NCH)]
    fill = [0] * NCH
    order = sorted(comps.items(),
                   key=lambda kv: (not has_coll[kv[0]], -len(kv[1])))
    for root, qs in order:
        for bi in range(NCH):
            if fill[bi] + len(qs) <= QUADS // NCH:
                bins[bi].extend(qs)
                fill[bi] += len(qs)
                break
        else:
            raise RuntimeError("quad component packing overflow")

    # Event layout: chunk c, position = quad slot * 4 + d.
    qpos = {}
    xg = np.zeros((S_C,), np.int64)     # gather index into per-core x rows
    for c, qs in enumerate(bins):
        for j, qi in enumerate(qs):
            qpos[qi] = (c, j)
            slot, tm, _ = quads[qi]
            base = c * P + j * D
            xg[base:base + D] = slot * (T * MOD * D) + tm * D + np.arange(D)
    # pads: point at x rows 0..3 (garbage, never scattered)

    # Collision gather: groups sorted, events in time order within group.
    scatter_single = []   # (b, id, chunk, pos)
    scatter_coll = []     # (b, id, gather_slot_of_first)
    gsrc = []             # (chunk, pos) per gather slot
    gid_of_slot = []
    for gi, ((slot, nid), members_) in enumerate(sorted(groups.items())):
        if len(members_) == 1:
            qi, d = members_[0]
            c, j = qpos[qi]
            scatter_single.append((pats[slot], nid, c, j * D + d))
            continue
        members_s = sorted(members_, key=lambda md: (quads[md[0]][1], md[1]))
        scatter_coll.append((pats[slot], nid, len(gsrc)))
        for qi, d in members_s:
            c, j = qpos[qi]
            gsrc.append((c, j * D + d))
            gid_of_slot.append(gi)
    n_coll = len(gsrc)
    assert n_coll <= P, f"collision events {n_coll} > {P}"
    ncc = max((c + 1 for (c, _q) in gsrc), default=1)
    maxg = max((len(m) for m in groups.values()), default=1)
    steps = max(1, math.ceil(math.log2(max(maxg, 2))))

    psel = np.zeros((NCH, P, P), np.float16)
    for p, (c, q) in enumerate(gsrc):
        psel[c, q, p] = 1.0
    g0 = np.zeros((P, P), np.float16)
    ga = np.array(gid_of_slot + [-1 - i for i in range(P - n_coll)])
    g0[ga[:, None] == ga[None, :]] = 1.0
    sc = np.zeros((steps, P, P), np.float16)
    scb = np.zeros((steps, P), np.float16)
    for s in range(steps):
        dist = 1 << s
        for p in range(P):
            if p + dist < n_coll and ga[p] == ga[p + dist]:
                sc[s, p + dist, p] = 1.0
            else:
                scb[s, p] = 1.0

    xe = x[pats].reshape(BP * T * MOD * D, WD)[xg].T.astype(np.float16)  # [WD,S_C]
    return (xe, psel, g0, sc, scb, steps, ncc, scatter_single, scatter_coll)


def _host_prep(inputs):
    x = np.asarray(inputs["input"], np.float32).reshape(B, T * MOD * D, WD)
    mask = np.asarray(inputs["mask"])
    valid_mod = np.asarray(inputs["valid_mod"])
    node_ids = np.asarray(inputs["node_ids"])
    demo = np.asarray(inputs["demo"], np.float32)
    W1 = np.asarray(inputs["W1"], np.float32)
    b1 = np.asarray(inputs["b1"], np.float32)
    W2 = np.asarray(inputs["W2"], np.float32)
    b2 = np.asarray(inputs["b2"], np.float32)
    W3 = np.asarray(inputs["W3"], np.float32)
    b3 = np.asarray(inputs["b3"], np.float32)
    We = np.asarray(inputs["We"], np.float32)
    be = np.asarray(inputs["be"], np.float32)
    Wa = np.asarray(inputs["Wa"], np.float32)
    ba = np.asarray(inputs["ba"], np.float32)
    init_mem = np.asarray(inputs["init_mem"], np.float32)
    assert not (be.any() or ba.any()), "nonzero gate biases unsupported"

    # ---- shared wewaA ----
    wA = np.zeros((P, WA_COLS), np.float16)
    WeWa = np.concatenate([We.reshape(2, P, MEM) * 0.5,
                           Wa.reshape(2, P, MEM)], axis=2)  # [2,128,512]
    wA[:, WA_G:WA_G + 1024] = WeWa.transpose(1, 0, 2).reshape(P, 1024)
    dpat = np.arange(P) % D
    for k in (1, 2, 3):
        msk = ((dpat + k) <= 3).astype(np.float32)
        ca = -(0.5 ** (k + 1)) * msk
        cs = (0.5 ** k) * msk
        Sth = np.zeros((P, P), np.float32)
        Sa = np.zeros((P, P), np.float32)
        for p in range(P - k):
            Sth[p + k, p] = ca[p]
            Sa[p + k, p] = cs[p]
        wA[:, WA_STH + (k - 1) * P:WA_STH + k * P] = Sth.astype(np.float16)
        wA[:, WA_SA + (k - 1) * P:WA_SA + k * P] = Sa.astype(np.float16)
        wA[0, WA_BTH + (k - 1) * P:WA_BTH + k * P] = (1.0 + ca).astype(np.float16)
    wA[0, WA_ONES:WA_ONES + MEM] = 1.0

    # ---- per-core prep ----
    gvalid = (mask[:, :, None] > 0) & (valid_mod > 0)   # [B, T, MOD]
    members = _assign_patients(gvalid)
    cores = []
    for core in range(N_CORES):
        pats = members[core]
        cores.append(_prep_core(x, node_ids, gvalid[pats], pats))
    steps = max(c[5] for c in cores)
    ncc = max(c[6] for c in cores)

    W2P = W2.reshape(4, P, DEMO).transpose(1, 0, 2).reshape(P, 4 * DEMO)
    wb_cols = WB_SC + 2 * steps * P
    in_maps = []
    scat = []
    init_d = init_mem.reshape(1, MEM).astype(np.float32)
    for core in range(N_CORES):
        (xe, psel, g0, sc, scb, _st, _ncc, ssing, scoll) = cores[core]
        pats = members[core]
        wBc = np.zeros((P, wb_cols), np.float16)
        wBc[:, WB_PSEL:WB_PSEL + NCH * P] = psel.transpose(1, 0, 2).reshape(P, NCH * P)
        wBc[:, WB_G0:WB_G0 + P] = g0
        wBc[0:DEMO, WB_W1:WB_W1 + 512] = W1.astype(np.float16)
        wBc[:, WB_W2P:WB_W2P + 256] = W2P.astype(np.float16)
        wBc[0:DEMO, WB_W3B:WB_W3B + 256] = W3.astype(np.float16)
        wBc[DEMO, WB_W3B:WB_W3B + 256] = b3.astype(np.float16)
        wBc[0:DEMO, WB_DT:WB_DT + BP] = demo[pats].T.astype(np.float16)
        f32block = np.zeros((P, 8), np.float32)
        f32block[:, 0:4] = b1.reshape(4, P).T
        f32block[0:DEMO, 4] = b2
        wBc[:, WB_F32:WB_F32 + 16] = f32block.view(np.float16)
        for s in range(steps):
            if s < sc.shape[0]:
                wBc[:, WB_SC + 2 * s * P:WB_SC + (2 * s + 1) * P] = sc[s]
                wBc[0, WB_SC + (2 * s + 1) * P:WB_SC + (2 * s + 2) * P] = scb[s]
            else:
                # extra doubling steps are harmless identity steps (mask=0)
                wBc[0, WB_SC + (2 * s + 1) * P:WB_SC + (2 * s + 2) * P] = 1.0
        in_maps.append({
            "initd": init_d,
            "xT0": np.ascontiguousarray(xe[0:P]),
            "xT1": np.ascontiguousarray(xe[P:2 * P]),
            "wewaA": wA, "wewaB": wBc,
        })
        scat.append((ssing, scoll))
    return in_maps, members, scat, (steps, ncc)


def _assemble(res, members, scat, init_mem):
    out = np.empty((B, N_NODES, MEM), np.float32)
    for core in range(N_CORES):
        r = res.results[core]
        block = np.asarray(r["out"]).reshape(BP, N_NODES, MEM)
        rows = np.asarray(r["rows"]).astype(np.float32)
        ssing, scoll = scat[core]
        for slot, b in enumerate(members[core]):
            out[b] = block[slot]
        for (b, nid, c, pos) in ssing:
            out[b, nid] = rows[c * P + pos]
        for (b, nid, gslot) in scoll:
            out[b, nid] = rows[NCH * P + gslot]
        for slot, b in enumerate(members[core]):
            out[b, 0] = rows[(NCH + 1) * P + slot]
    return out


def get_nc(cfg=(1, 1)):
    if cfg not in _NC_CACHE:
        _NC_CACHE[cfg] = _build_nc(*cfg)
    return _NC_CACHE[cfg]


def run_cores(inputs, trace=False):
    in_maps, members, scat, cfg = _host_prep(inputs)
    nc = get_nc(cfg)
    res = bass_utils.run_bass_kernel_spmd(
        nc, in_maps, core_ids=list(range(N_CORES)), trace=trace)
    init_mem = np.asarray(inputs["init_mem"], np.float32)
    return _assemble(res, members, scat, init_mem), res


def kernel(**inputs) -> np.ndarray:
    return run_cores(inputs)[0]


if __name__ == "__main__":
    ref = {}
    exec(open("/root/problem/reference.py").read(), ref)
    inputs = {k: np.asarray(v) for k, v in ref["setup_inputs"]().items()}
    got = kernel(**inputs)
    want = np.asarray(ref["reference"](**inputs))
    err = np.abs(got - want).max() / np.abs(want).max()
    print("rel err:", err)
